# revision 2
# baseline (speedup 1.0000x reference)
"""Trainium2 8-core kernel for nn_BasicSubGraphLearner.

reference semantics:
  ctx[p,n,d] = weight[p,d] * x[n,d], row-normalized over d
  adj = einsum('pnd,pmd->nm', ctx, ctx) / P          (N x N)
  adj = adj * edge_mask;  adj = where(adj > 0.5, adj, 0);  zero diagonal

Device strategy (row-sharded, per sharding hint): the N x N similarity is
a (N, P*D) @ (P*D, N) gram matrix. Each of the 8 cores computes a
1024-row slice: stationary = its (2048, 1024) slice of the K-major
context matrix, moving = all 8192 columns streamed in 512-wide chunks,
bf16 matmuls accumulating f32 in PSUM over 16 k-tiles, epsilon threshold
fused into PSUM evacuation. Host does the cheap O(N*D) normalization /
layout, and applies the edge mask by gather (equivalent to dense
mask-then-threshold since threshold(0) = 0).
"""

import sys

if "/opt/trn_rl_repo" not in sys.path:
    sys.path.insert(0, "/opt/trn_rl_repo")

import numpy as np
import ml_dtypes

from concourse import bacc, bass, tile, mybir
from concourse.bass_utils import run_bass_kernel_spmd

N = 8192
D = 256
P = 8
EPSILON = 0.5
N_CORES = 8
K = P * D               # 2048 contraction
KT = K // 128           # 16 k-tiles
M_PER_CORE = N // N_CORES   # 1024 rows per core
MT = M_PER_CORE // 128      # 8 m-tiles
NCHUNK = 512
NJ = N // NCHUNK            # 16 column chunks

_BF16 = mybir.dt.bfloat16
_F32 = mybir.dt.float32


def build_program(n=N, k=K, m_per_core=M_PER_CORE, nchunk=NCHUNK):
    """SPMD program: out[m_per_core, n] = threshold(kxm.T @ kxn)."""
    kt = k // 128
    mt = m_per_core // 128
    nj = n // nchunk
    nc = bacc.Bacc("TRN2", target_bir_lowering=False, debug=False,
                   num_devices=N_CORES)
    kxm = nc.dram_tensor("kxm", [k, m_per_core], _BF16, kind="ExternalInput").ap()
    kxn = nc.dram_tensor("kxn", [k, n], _BF16, kind="ExternalInput").ap()
    out = nc.dram_tensor("out", [m_per_core, n], _F32, kind="ExternalOutput").ap()

    kxm_t = kxm.rearrange("(t p) m -> p t m", p=128)
    kxn_t = kxn.rearrange("(t p) n -> p t n", p=128)

    with tile.TileContext(nc) as tc:
        with (
            tc.tile_pool(name="apool", bufs=1) as apool,
            tc.tile_pool(name="bpool", bufs=2) as bpool,
            tc.tile_pool(name="opool", bufs=4) as opool,
            tc.tile_pool(name="psum", bufs=4, space=bass.MemorySpace.PSUM) as pp,
        ):
            # stationary: whole row-block resident in SBUF
            a = apool.tile([128, kt, m_per_core], _BF16)
            for t in range(kt):
                nc.sync.dma_start(out=a[:, t, :], in_=kxm_t[:, t, :])

            for j in range(nj):
                b = bpool.tile([128, kt, nchunk], _BF16, tag="b")
                for t in range(kt):
                    nc.sync.dma_start(
                        out=b[:, t, :], in_=kxn_t[:, t, j * nchunk:(j + 1) * nchunk]
                    )
                for m in range(mt):
                    ps = pp.tile([128, nchunk], _F32, tag="ps")
                    for t in range(kt):
                        nc.tensor.matmul(
                            ps[:],
                            a[:, t, m * 128:(m + 1) * 128],
                            b[:, t, :],
                            start=(t == 0),
                            stop=(t == kt - 1),
                        )
                    o = opool.tile([128, nchunk], _F32, tag="o")
                    msk = opool.tile([128, nchunk], _F32, tag="msk")
                    # where(v > eps, v, 0) == (v > eps) * v
                    nc.vector.tensor_scalar(
                        msk[:], ps[:], EPSILON, None, op0=mybir.AluOpType.is_gt
                    )
                    nc.vector.tensor_tensor(
                        o[:], ps[:], msk[:], op=mybir.AluOpType.mult
                    )
                    nc.sync.dma_start(
                        out=out[m * 128:(m + 1) * 128, j * nchunk:(j + 1) * nchunk],
                        in_=o[:],
                    )
    nc.compile()
    return nc


_CACHED = {}


def _get_program(key, *args):
    if key not in _CACHED:
        _CACHED[key] = build_program(*args)
    return _CACHED[key]


def _preprocess(x, weight):
    """ctxN: (K, N) bf16, K-index = p*D + d, rows L2-normalized over d."""
    x = np.asarray(x, np.float32)
    w = np.asarray(weight, np.float32)
    ctx = w[:, None, :] * x[None, :, :]                      # (P, N, D)
    norm = np.sqrt((ctx * ctx).sum(-1, keepdims=True))
    ctx /= np.maximum(norm, 1e-12)
    ctx *= np.float32(1.0 / np.sqrt(P))                      # fold 1/P in
    ctxn = ctx.transpose(0, 2, 1).reshape(K, N)              # (p*D+d, n)
    return np.ascontiguousarray(ctxn).astype(ml_dtypes.bfloat16)


def kernel(x, weight, full_edge_index, _trace=False):
    ctxn = _preprocess(x, weight)
    nc = _get_program("main")

    in_maps = [
        {"kxm": np.ascontiguousarray(ctxn[:, c * M_PER_CORE:(c + 1) * M_PER_CORE]),
         "kxn": ctxn}
        for c in range(N_CORES)
    ]
    res = run_bass_kernel_spmd(nc, in_maps, list(range(N_CORES)), trace=_trace)
    thr = np.concatenate([res.results[c]["out"] for c in range(N_CORES)], axis=0)

    # edge-mask by gather: threshold(adj*mask) == mask-gather of threshold(adj)
    e0 = np.asarray(full_edge_index[0])
    e1 = np.asarray(full_edge_index[1])
    keep = e0 != e1                                          # RemoveSelfLoop
    result = np.zeros((N, N), np.float32)
    result[e0[keep], e1[keep]] = thr[e0[keep], e1[keep]]
    if _trace:
        return result, res
    return result
